# revision 24
# baseline (speedup 1.0000x reference)
"""LoRA basis-bank kernel for 8 TRN2 NeuronCores.

Math (per batch b):
    A_mixed  = sum_k alpha[b,k] * A_bank[k]        # [R, DIN]
    B_mixedT = sum_k alpha[b,k] * B_bank[k].T      # [R, DOUT]
    z        = h[b] @ A_mixed.T                    # [S, R]
    delta[b] = z @ B_mixedT                        # [S, DOUT]

Sharding: data-parallel over batch, 1 batch per core; banks replicated.

Host-side layout prep (no arithmetic): h shard uploaded transposed
(hT[i, s]) bf16; B_bank uploaded as [K, R, DOUT]; alpha expanded into a
[K*R, R] block-diagonal placement matrix; banks bf16.

v9 device dataflow — s-halved software pipeline so delta stores for the
first half of S overlap the DMA loads of the second half (HBM is the
serial resource; this keeps it continuously busy):
  - hT as 32 resident [128, 1024] half-tiles (2KB descriptors), split
    SP (a_flat + c0-9 halves) / ACT (mix + c10-15 halves + b_flat).
  - mm1 per half: 2 matmuls per chunk at PE column positions 0/64 into
    one PSUM bank per half; JIT A_mixT mixing one chunk ahead.
  - After half A: zT-A cast, B_mixedT mix + replication, then mm2
    pairs for s-tiles 0-7 interleaved with mm1-B chunks as the B
    half-tiles stream in.
  - mm2 s-tile pairs hit alternating PE row bands 0/64 (concurrent
    quadrants); casts per [128,1024] unit alternate DVE/ACT; stores
    ride the SP queue.
"""

import ml_dtypes
import numpy as np

import concourse.bacc as bacc
import concourse.bass as bass
import concourse.mybir as mybir
import concourse.tile as tile
from concourse.bass_utils import run_bass_kernel_spmd

B, S, K, R, DIN, DOUT = 8, 2048, 16, 16, 2048, 2048
KR = K * R  # 256
F32 = mybir.dt.float32
BF16 = mybir.dt.bfloat16

_cache = {}

# Per-half chunk processing order ~= DMA arrival order (SP c0-9 after
# a_flat, ACT c10-15 after mix/banks).
_ARRIVE_A = [0, 1, 10, 2, 11, 3, 12, 4, 13, 5, 14, 6, 15, 7, 8, 9]
_ARRIVE_B = [10, 0, 11, 1, 12, 2, 13, 3, 14, 4, 15, 5, 6, 7, 8, 9]
_PAIRS_A = [(0, 4), (1, 5), (2, 6), (3, 7)]
_PAIRS_B = [(8, 12), (9, 13), (10, 14), (11, 15)]


def _build_nc():
    nc = bacc.Bacc("TRN2", target_bir_lowering=False,
                   enable_partition_id=False)

    ht_d = nc.dram_tensor("hbT", [DIN, S], BF16, kind="ExternalInput")
    mix_d = nc.dram_tensor("mix", [KR, R], BF16, kind="ExternalInput")
    a_d = nc.dram_tensor("a_flat", [KR, DIN], BF16, kind="ExternalInput")
    bt_d = nc.dram_tensor("bt_flat", [KR, DOUT], BF16, kind="ExternalInput")
    out_d = nc.dram_tensor("delta", [S, DOUT], BF16, kind="ExternalOutput")

    NCH = DIN // 128  # 16 chunks along DIN
    HS = S // 2       # 1024 columns per s-half
    with tile.TileContext(nc) as tc:
        with (
            tc.tile_pool(name="hT", bufs=1) as hTp,
            tc.tile_pool(name="banks", bufs=1) as bankp,
            tc.tile_pool(name="mixed", bufs=1) as mixp,
            tc.tile_pool(name="dout", bufs=4) as dp,
        ):
            hTs = [[None] * NCH, [None] * NCH]
            for half in range(2):
                for c in range(NCH):
                    hTs[half][c] = hTp.tile([128, HS], BF16,
                                            tag=f"hT{c}_{half}",
                                            name=f"hT{c}_{half}")

            # ---- SP queue: a_flat then c0-9 A-halves then B-halves ----
            a_sb = []
            for h2 in range(2):
                a_t = bankp.tile([128, DIN], BF16, tag=f"a{h2}")
                nc.sync.dma_start(a_t[:], a_d[h2 * 128:(h2 + 1) * 128, :])
                a_sb.append(a_t)
            for half in range(2):
                for c in range(10):
                    nc.sync.dma_start(
                        hTs[half][c][:],
                        ht_d[c * 128:(c + 1) * 128,
                             half * HS:(half + 1) * HS])

            # ---- ACT queue: mix, c10-15 A-halves, b_flat, B-halves ----
            m_sb, b_sb = [], []
            for h2 in range(2):
                m_t = bankp.tile([128, R], BF16, tag=f"m{h2}")
                nc.scalar.dma_start(m_t[:], mix_d[h2 * 128:(h2 + 1) * 128, :])
                m_sb.append(m_t)
            for c in range(10, NCH):
                nc.scalar.dma_start(
                    hTs[0][c][:], ht_d[c * 128:(c + 1) * 128, 0:HS])
            for h2 in range(2):
                b_t = bankp.tile([128, DOUT], BF16, tag=f"b{h2}")
                nc.scalar.dma_start(b_t[:], bt_d[h2 * 128:(h2 + 1) * 128, :])
                b_sb.append(b_t)
            for c in range(10, NCH):
                nc.scalar.dma_start(
                    hTs[1][c][:], ht_d[c * 128:(c + 1) * 128, HS:S])

            bmix2 = mixp.tile([128, DOUT], BF16, tag="bmix2")
            zt2 = [mixp.tile([128, 512], BF16, tag=f"zt2_{i}", name=f"zt2_{i}")
                   for i in range(2)]
            amixT = [mixp.tile([128, R], BF16, tag=f"amixT{c}",
                               name=f"amixT{c}") for c in range(NCH)]

            with tc.tile_pool(name="psz", bufs=1, space="PSUM") as pszp:
                zt_ps = [pszp.tile([128, 512], F32, tag=f"ztall{i}",
                                   name=f"zt_ps{i}") for i in range(2)]

                def mm1_chunk(half, c, first, last):
                    for sc2 in range(2):
                        po = 64 * sc2
                        nc.tensor.matmul(
                            zt_ps[half][po:po + R, :], amixT[c][:],
                            hTs[half][c][:, sc2 * 512:(sc2 + 1) * 512],
                            start=first, stop=last,
                            skip_group_check=True)

                with tc.tile_pool(name="psm", bufs=1, space="PSUM") as psmp:
                    def amix_chunk(c, slot):
                        csl = slice(c * 128, (c + 1) * 128)
                        pat = psmp.tile([128, R], F32, tag=f"pat{slot}",
                                        name=f"pat{slot}")
                        nc.tensor.matmul(pat[:], a_sb[0][:, csl], m_sb[0][:],
                                         start=True, stop=False)
                        nc.tensor.matmul(pat[:], a_sb[1][:, csl], m_sb[1][:],
                                         start=False, stop=True)
                        nc.vector.tensor_copy(amixT[c][:], pat[:])

                    def bmix_chunk(c4):
                        sl = slice(c4 * 512, (c4 + 1) * 512)
                        pmix = psmp.tile([R, 512], F32, tag="pmix")
                        nc.tensor.matmul(pmix[:], m_sb[0][:], b_sb[0][:, sl],
                                         start=True, stop=False)
                        nc.tensor.matmul(pmix[:], m_sb[1][:], b_sb[1][:, sl],
                                         start=False, stop=True)
                        nc.vector.tensor_copy(bmix2[0:R, sl], pmix[:])

                    # JIT mixing + mm1 half A, one chunk ahead
                    amix_chunk(_ARRIVE_A[0], 0)
                    for idx in range(NCH - 1):
                        amix_chunk(_ARRIVE_A[idx + 1], (idx + 1) % 2)
                        mm1_chunk(0, _ARRIVE_A[idx], first=(idx == 0),
                                  last=False)
                    mm1_chunk(0, _ARRIVE_A[NCH - 1], first=False, last=True)
                    nc.vector.tensor_copy(zt2[0][:], zt_ps[0][:])
                    # B_mixedT after half-A mm1 (b_flat lands mid-phase-A)
                    for c4 in range(4):
                        bmix_chunk(c4)
                    nc.vector.tensor_copy(bmix2[64:64 + R, :], bmix2[0:R, :])

                with tc.tile_pool(name="psd", bufs=3, space="PSUM") as psdp:
                    def unit(st, hp):
                        sc = st // 4
                        po = 64 * (sc % 2)
                        scol = slice((st % 4) * 128, (st % 4) * 128 + 128)
                        dps = psdp.tile([128, 1024], F32, tag="dps")
                        for q in range(2):
                            osl = slice((2 * hp + q) * 512,
                                        (2 * hp + q) * 512 + 512)
                            nc.tensor.matmul(
                                dps[:, q * 512:(q + 1) * 512],
                                zt2[sc // 2][po:po + R, scol],
                                bmix2[po:po + R, osl])
                        return dps

                    def pair(stA, stB):
                        # one cast engine per tile: the store then waits
                        # on a single engine's sem chain (DVE for A,
                        # ACT for B) instead of both
                        dsbA = dp.tile([128, DOUT], BF16, tag="d",
                                       name="dsbA")
                        dsbB = dp.tile([128, DOUT], BF16, tag="d",
                                       name="dsbB")
                        for hp in range(2):
                            dpsA = unit(stA, hp)
                            dpsB = unit(stB, hp)
                            dcol = slice(hp * 1024, (hp + 1) * 1024)
                            nc.vector.tensor_copy(dsbA[:, dcol], dpsA[:])
                            nc.scalar.copy(dsbB[:, dcol], dpsB[:])
                        nc.sync.dma_start(
                            out_d[stA * 128:stA * 128 + 128, :], dsbA[:])
                        nc.sync.dma_start(
                            out_d[stB * 128:stB * 128 + 128, :], dsbB[:])

                    # overlap: half-A delta pairs interleaved with half-B
                    # mm1 chunks as the B half-tiles stream in
                    for g in range(4):
                        pair(*_PAIRS_A[g])
                        for j in range(4):
                            idx = 4 * g + j
                            mm1_chunk(1, _ARRIVE_B[idx], first=(idx == 0),
                                      last=(idx == NCH - 1))
                    nc.vector.tensor_copy(zt2[1][:], zt_ps[1][:])
                    for g in range(4):
                        pair(*_PAIRS_B[g])

    nc.compile()
    return nc


def _in_maps(h, alpha, A_bank, B_bank):
    a_flat = np.ascontiguousarray(
        A_bank.reshape(KR, DIN)).astype(ml_dtypes.bfloat16)
    bt_flat = np.ascontiguousarray(
        B_bank.transpose(0, 2, 1).reshape(KR, DOUT)).astype(ml_dtypes.bfloat16)
    eye = np.eye(R, dtype=np.float32)
    maps = []
    for b in range(B):
        mix = np.kron(alpha[b].astype(np.float32).reshape(K, 1),
                      eye).astype(ml_dtypes.bfloat16)
        hT = np.ascontiguousarray(
            np.asarray(h[b]).T).astype(ml_dtypes.bfloat16)
        maps.append({
            "hbT": hT,
            "mix": np.ascontiguousarray(mix),
            "a_flat": a_flat,
            "bt_flat": bt_flat,
        })
    return maps


def _run(inputs, trace=False):
    if "nc" not in _cache:
        _cache["nc"] = _build_nc()
    nc = _cache["nc"]
    maps = _in_maps(inputs["h"], inputs["alpha"], inputs["A_bank"],
                    inputs["B_bank"])
    res = run_bass_kernel_spmd(nc, maps, core_ids=list(range(B)), trace=trace)
    out = np.stack([res.results[b]["delta"] for b in range(B)], axis=0)
    return out.astype(np.float32), res


def kernel(**inputs):
    out, _ = _run(inputs, trace=False)
    return out


# revision 25
# speedup vs baseline: 1.0294x; 1.0294x over previous
"""LoRA basis-bank kernel for 8 TRN2 NeuronCores.

Math (per batch b):
    A_mixed  = sum_k alpha[b,k] * A_bank[k]        # [R, DIN]
    B_mixedT = sum_k alpha[b,k] * B_bank[k].T      # [R, DOUT]
    z        = h[b] @ A_mixed.T                    # [S, R]
    delta[b] = z @ B_mixedT                        # [S, DOUT]

Sharding: data-parallel over batch, 1 batch per core; banks replicated.

Host-side layout prep (no arithmetic): h shard uploaded transposed
(hT[i, s]) bf16; B_bank uploaded as [K, R, DOUT]; alpha expanded into a
[K*R, R] block-diagonal placement matrix; banks bf16.

v9 device dataflow — s-halved software pipeline so delta stores for the
first half of S overlap the DMA loads of the second half (HBM is the
serial resource; this keeps it continuously busy):
  - hT as 32 resident [128, 1024] half-tiles (2KB descriptors), split
    SP (a_flat + c0-9 halves) / ACT (mix + c10-15 halves + b_flat).
  - mm1 per half: 2 matmuls per chunk at PE column positions 0/64 into
    one PSUM bank per half; JIT A_mixT mixing one chunk ahead.
  - After half A: zT-A cast, B_mixedT mix + replication, then mm2
    pairs for s-tiles 0-7 interleaved with mm1-B chunks as the B
    half-tiles stream in.
  - mm2 s-tile pairs hit alternating PE row bands 0/64 (concurrent
    quadrants); casts per [128,1024] unit alternate DVE/ACT; stores
    ride the SP queue.
"""

import ml_dtypes
import numpy as np

import concourse.bacc as bacc
import concourse.bass as bass
import concourse.mybir as mybir
import concourse.tile as tile
from concourse.bass_utils import run_bass_kernel_spmd

B, S, K, R, DIN, DOUT = 8, 2048, 16, 16, 2048, 2048
KR = K * R  # 256
F32 = mybir.dt.float32
BF16 = mybir.dt.bfloat16

_cache = {}

# Per-half chunk processing order ~= DMA arrival order (SP c0-9 after
# a_flat, ACT c10-15 after mix/banks).
_ARRIVE_A = [0, 1, 10, 2, 11, 3, 12, 4, 13, 5, 14, 6, 15, 7, 8, 9]
_ARRIVE_B = [10, 0, 11, 1, 12, 2, 13, 3, 14, 4, 15, 5, 6, 7, 8, 9]
_PAIRS_A = [(0, 4), (1, 5), (2, 6), (3, 7)]
_PAIRS_B = [(8, 12), (9, 13), (10, 14), (11, 15)]


def _build_nc():
    nc = bacc.Bacc("TRN2", target_bir_lowering=False,
                   enable_partition_id=False)

    ht_d = nc.dram_tensor("hbT", [DIN, S], BF16, kind="ExternalInput")
    mix_d = nc.dram_tensor("mix", [KR, R], BF16, kind="ExternalInput")
    a_d = nc.dram_tensor("a_flat", [KR, DIN], BF16, kind="ExternalInput")
    bt_d = nc.dram_tensor("bt_flat", [KR, DOUT], BF16, kind="ExternalInput")
    out_d = nc.dram_tensor("delta", [S, DOUT], BF16, kind="ExternalOutput")

    NCH = DIN // 128  # 16 chunks along DIN
    HS = S // 2       # 1024 columns per s-half
    with tile.TileContext(nc) as tc:
        with (
            tc.tile_pool(name="hT", bufs=1) as hTp,
            tc.tile_pool(name="banks", bufs=1) as bankp,
            tc.tile_pool(name="mixed", bufs=1) as mixp,
            tc.tile_pool(name="dout", bufs=3) as dp,
        ):
            hTs = [[None] * NCH, [None] * NCH]
            for half in range(2):
                for c in range(NCH):
                    hTs[half][c] = hTp.tile([128, HS], BF16,
                                            tag=f"hT{c}_{half}",
                                            name=f"hT{c}_{half}")

            # ---- SP queue: a_flat then c0-9 A-halves then B-halves ----
            a_sb = []
            for h2 in range(2):
                a_t = bankp.tile([128, DIN], BF16, tag=f"a{h2}")
                nc.sync.dma_start(a_t[:], a_d[h2 * 128:(h2 + 1) * 128, :])
                a_sb.append(a_t)
            for half in range(2):
                for c in range(10):
                    nc.sync.dma_start(
                        hTs[half][c][:],
                        ht_d[c * 128:(c + 1) * 128,
                             half * HS:(half + 1) * HS])

            # ---- ACT queue: mix, c10-15 A-halves, b_flat, B-halves ----
            m_sb, b_sb = [], []
            for h2 in range(2):
                m_t = bankp.tile([128, R], BF16, tag=f"m{h2}")
                nc.scalar.dma_start(m_t[:], mix_d[h2 * 128:(h2 + 1) * 128, :])
                m_sb.append(m_t)
            for c in range(10, NCH):
                nc.scalar.dma_start(
                    hTs[0][c][:], ht_d[c * 128:(c + 1) * 128, 0:HS])
            for h2 in range(2):
                b_t = bankp.tile([128, DOUT], BF16, tag=f"b{h2}")
                nc.scalar.dma_start(b_t[:], bt_d[h2 * 128:(h2 + 1) * 128, :])
                b_sb.append(b_t)
            for c in range(10, NCH):
                nc.scalar.dma_start(
                    hTs[1][c][:], ht_d[c * 128:(c + 1) * 128, HS:S])

            bmix2 = mixp.tile([128, DOUT], BF16, tag="bmix2")
            zt2 = [mixp.tile([128, 512], BF16, tag=f"zt2_{i}", name=f"zt2_{i}")
                   for i in range(2)]
            amixT = [mixp.tile([128, R], BF16, tag=f"amixT{c}",
                               name=f"amixT{c}") for c in range(NCH)]

            with tc.tile_pool(name="psz", bufs=1, space="PSUM") as pszp:
                zt_ps = [pszp.tile([128, 512], F32, tag=f"ztall{i}",
                                   name=f"zt_ps{i}") for i in range(2)]

                def mm1_chunk(half, c, first, last):
                    for sc2 in range(2):
                        po = 64 * sc2
                        nc.tensor.matmul(
                            zt_ps[half][po:po + R, :], amixT[c][:],
                            hTs[half][c][:, sc2 * 512:(sc2 + 1) * 512],
                            start=first, stop=last,
                            skip_group_check=True)

                with tc.tile_pool(name="psm", bufs=1, space="PSUM") as psmp:
                    def amix_chunk(c, slot):
                        csl = slice(c * 128, (c + 1) * 128)
                        pat = psmp.tile([128, R], F32, tag=f"pat{slot}",
                                        name=f"pat{slot}")
                        nc.tensor.matmul(pat[:], a_sb[0][:, csl], m_sb[0][:],
                                         start=True, stop=False)
                        nc.tensor.matmul(pat[:], a_sb[1][:, csl], m_sb[1][:],
                                         start=False, stop=True)
                        nc.vector.tensor_copy(amixT[c][:], pat[:])

                    def bmix_chunk(c4):
                        sl = slice(c4 * 512, (c4 + 1) * 512)
                        pmix = psmp.tile([R, 512], F32, tag="pmix")
                        nc.tensor.matmul(pmix[:], m_sb[0][:], b_sb[0][:, sl],
                                         start=True, stop=False)
                        nc.tensor.matmul(pmix[:], m_sb[1][:], b_sb[1][:, sl],
                                         start=False, stop=True)
                        nc.vector.tensor_copy(bmix2[0:R, sl], pmix[:])

                    # JIT mixing + mm1 half A, one chunk ahead
                    amix_chunk(_ARRIVE_A[0], 0)
                    for idx in range(NCH - 1):
                        amix_chunk(_ARRIVE_A[idx + 1], (idx + 1) % 2)
                        mm1_chunk(0, _ARRIVE_A[idx], first=(idx == 0),
                                  last=False)
                    mm1_chunk(0, _ARRIVE_A[NCH - 1], first=False, last=True)
                    nc.vector.tensor_copy(zt2[0][:], zt_ps[0][:])
                    # B_mixedT after half-A mm1 (b_flat lands mid-phase-A)
                    for c4 in range(4):
                        bmix_chunk(c4)
                    nc.vector.tensor_copy(bmix2[64:64 + R, :], bmix2[0:R, :])

                with tc.tile_pool(name="psd", bufs=3, space="PSUM") as psdp:
                    def unit(st, hp):
                        sc = st // 4
                        po = 64 * (sc % 2)
                        scol = slice((st % 4) * 128, (st % 4) * 128 + 128)
                        dps = psdp.tile([128, 1024], F32, tag="dps")
                        for q in range(2):
                            osl = slice((2 * hp + q) * 512,
                                        (2 * hp + q) * 512 + 512)
                            nc.tensor.matmul(
                                dps[:, q * 512:(q + 1) * 512],
                                zt2[sc // 2][po:po + R, scol],
                                bmix2[po:po + R, osl])
                        return dps

                    def pair(stA, stB):
                        dsbA = dp.tile([128, DOUT], BF16, tag="d",
                                       name="dsbA")
                        dsbB = dp.tile([128, DOUT], BF16, tag="d",
                                       name="dsbB")
                        for hp in range(2):
                            dpsA = unit(stA, hp)
                            dpsB = unit(stB, hp)
                            dcol = slice(hp * 1024, (hp + 1) * 1024)
                            if hp == 0:
                                nc.vector.tensor_copy(dsbA[:, dcol], dpsA[:])
                                nc.scalar.copy(dsbB[:, dcol], dpsB[:])
                            else:
                                nc.scalar.copy(dsbA[:, dcol], dpsA[:])
                                nc.vector.tensor_copy(dsbB[:, dcol], dpsB[:])
                        nc.sync.dma_start(
                            out_d[stA * 128:stA * 128 + 128, :], dsbA[:])
                        nc.sync.dma_start(
                            out_d[stB * 128:stB * 128 + 128, :], dsbB[:])

                    # overlap: half-A delta pairs interleaved with half-B
                    # mm1 chunks as the B half-tiles stream in
                    for g in range(4):
                        pair(*_PAIRS_A[g])
                        for j in range(4):
                            idx = 4 * g + j
                            mm1_chunk(1, _ARRIVE_B[idx], first=(idx == 0),
                                      last=(idx == NCH - 1))
                    nc.vector.tensor_copy(zt2[1][:], zt_ps[1][:])
                    for g in range(4):
                        pair(*_PAIRS_B[g])

    nc.compile()
    return nc


def _in_maps(h, alpha, A_bank, B_bank):
    a_flat = np.ascontiguousarray(
        A_bank.reshape(KR, DIN)).astype(ml_dtypes.bfloat16)
    bt_flat = np.ascontiguousarray(
        B_bank.transpose(0, 2, 1).reshape(KR, DOUT)).astype(ml_dtypes.bfloat16)
    eye = np.eye(R, dtype=np.float32)
    maps = []
    for b in range(B):
        mix = np.kron(alpha[b].astype(np.float32).reshape(K, 1),
                      eye).astype(ml_dtypes.bfloat16)
        hT = np.ascontiguousarray(
            np.asarray(h[b]).T).astype(ml_dtypes.bfloat16)
        maps.append({
            "hbT": hT,
            "mix": np.ascontiguousarray(mix),
            "a_flat": a_flat,
            "bt_flat": bt_flat,
        })
    return maps


def _run(inputs, trace=False):
    if "nc" not in _cache:
        _cache["nc"] = _build_nc()
    nc = _cache["nc"]
    maps = _in_maps(inputs["h"], inputs["alpha"], inputs["A_bank"],
                    inputs["B_bank"])
    res = run_bass_kernel_spmd(nc, maps, core_ids=list(range(B)), trace=trace)
    out = np.stack([res.results[b]["delta"] for b in range(B)], axis=0)
    return out.astype(np.float32), res


def kernel(**inputs):
    out, _ = _run(inputs, trace=False)
    return out


# revision 27
# speedup vs baseline: 1.0791x; 1.0483x over previous
"""LoRA basis-bank kernel for 8 TRN2 NeuronCores.

Math (per batch b):
    A_mixed  = sum_k alpha[b,k] * A_bank[k]        # [R, DIN]
    B_mixedT = sum_k alpha[b,k] * B_bank[k].T      # [R, DOUT]
    z        = h[b] @ A_mixed.T                    # [S, R]
    delta[b] = z @ B_mixedT                        # [S, DOUT]

Sharding: data-parallel over batch, 1 batch per core; banks replicated.

Host-side layout prep (no arithmetic): h shard uploaded transposed
(hT[i, s]) bf16; B_bank uploaded as [K, R, DOUT]; alpha expanded into a
[K*R, R] block-diagonal placement matrix; banks bf16.

v9 device dataflow — s-halved software pipeline so delta stores for the
first half of S overlap the DMA loads of the second half (HBM is the
serial resource; this keeps it continuously busy):
  - hT as 32 resident [128, 1024] half-tiles (2KB descriptors), split
    SP (a_flat + c0-9 halves) / ACT (mix + c10-15 halves + b_flat).
  - mm1 per half: 2 matmuls per chunk at PE column positions 0/64 into
    one PSUM bank per half; JIT A_mixT mixing one chunk ahead.
  - After half A: zT-A cast, B_mixedT mix + replication, then mm2
    pairs for s-tiles 0-7 interleaved with mm1-B chunks as the B
    half-tiles stream in.
  - mm2 s-tile pairs hit alternating PE row bands 0/64 (concurrent
    quadrants); casts per [128,1024] unit alternate DVE/ACT; stores
    ride the SP queue.
"""

import ml_dtypes
import numpy as np

import concourse.bacc as bacc
import concourse.bass as bass
import concourse.mybir as mybir
import concourse.tile as tile
from concourse.bass_utils import run_bass_kernel_spmd

B, S, K, R, DIN, DOUT = 8, 2048, 16, 16, 2048, 2048
KR = K * R  # 256
F32 = mybir.dt.float32
BF16 = mybir.dt.bfloat16

_cache = {}

# Per-half chunk processing order ~= DMA arrival order (SP c0-9 after
# a_flat, ACT c10-15 after mix/banks).
_ARRIVE_A = [0, 1, 10, 2, 11, 3, 12, 4, 13, 5, 14, 6, 15, 7, 8, 9]
_ARRIVE_B = [10, 0, 11, 1, 12, 2, 13, 3, 14, 4, 15, 5, 6, 7, 8, 9]
_PAIRS_A = [(0, 4), (1, 5), (2, 6), (3, 7)]
_PAIRS_B = [(8, 12), (9, 13), (10, 14), (11, 15)]


def _build_nc():
    nc = bacc.Bacc("TRN2", target_bir_lowering=False,
                   enable_partition_id=False)

    ht_d = nc.dram_tensor("hbT", [DIN, S], BF16, kind="ExternalInput")
    mix_d = nc.dram_tensor("mix", [KR, R], BF16, kind="ExternalInput")
    a_d = nc.dram_tensor("a_flat", [KR, DIN], BF16, kind="ExternalInput")
    bt_d = nc.dram_tensor("bt_flat", [KR, DOUT], BF16, kind="ExternalInput")
    out_d = nc.dram_tensor("delta", [S, DOUT], BF16, kind="ExternalOutput")

    NCH = DIN // 128  # 16 chunks along DIN
    HS = S // 2       # 1024 columns per s-half
    with tile.TileContext(nc) as tc:
        with (
            tc.tile_pool(name="hT", bufs=1) as hTp,
            tc.tile_pool(name="banks", bufs=1) as bankp,
            tc.tile_pool(name="mixed", bufs=1) as mixp,
            tc.tile_pool(name="dout", bufs=6) as dp,
        ):
            hTs = [[None] * NCH, [None] * NCH]
            for half in range(2):
                for c in range(NCH):
                    hTs[half][c] = hTp.tile([128, HS], BF16,
                                            tag=f"hT{c}_{half}",
                                            name=f"hT{c}_{half}")

            # ---- SP queue: a_flat then c0-9 A-halves then B-halves ----
            a_sb = []
            for h2 in range(2):
                a_t = bankp.tile([128, DIN], BF16, tag=f"a{h2}")
                nc.sync.dma_start(a_t[:], a_d[h2 * 128:(h2 + 1) * 128, :])
                a_sb.append(a_t)
            for half in range(2):
                for c in range(10):
                    nc.sync.dma_start(
                        hTs[half][c][:],
                        ht_d[c * 128:(c + 1) * 128,
                             half * HS:(half + 1) * HS])

            # ---- ACT queue: mix, c10-15 A-halves, b_flat, B-halves ----
            m_sb, b_sb = [], []
            for h2 in range(2):
                m_t = bankp.tile([128, R], BF16, tag=f"m{h2}")
                nc.scalar.dma_start(m_t[:], mix_d[h2 * 128:(h2 + 1) * 128, :])
                m_sb.append(m_t)
            for c in range(10, NCH):
                nc.scalar.dma_start(
                    hTs[0][c][:], ht_d[c * 128:(c + 1) * 128, 0:HS])
            for h2 in range(2):
                b_t = bankp.tile([128, DOUT], BF16, tag=f"b{h2}")
                nc.scalar.dma_start(b_t[:], bt_d[h2 * 128:(h2 + 1) * 128, :])
                b_sb.append(b_t)
            for c in range(10, NCH):
                nc.scalar.dma_start(
                    hTs[1][c][:], ht_d[c * 128:(c + 1) * 128, HS:S])

            bmix2 = mixp.tile([128, DOUT], BF16, tag="bmix2")
            zt2 = [mixp.tile([128, 512], BF16, tag=f"zt2_{i}", name=f"zt2_{i}")
                   for i in range(2)]
            amixT = [mixp.tile([128, R], BF16, tag=f"amixT{c}",
                               name=f"amixT{c}") for c in range(NCH)]

            with tc.tile_pool(name="psz", bufs=1, space="PSUM") as pszp:
                zt_ps = [pszp.tile([128, 512], F32, tag=f"ztall{i}",
                                   name=f"zt_ps{i}") for i in range(2)]

                def mm1_chunk(half, c, first, last):
                    for sc2 in range(2):
                        po = 64 * sc2
                        nc.tensor.matmul(
                            zt_ps[half][po:po + R, :], amixT[c][:],
                            hTs[half][c][:, sc2 * 512:(sc2 + 1) * 512],
                            start=first, stop=last,
                            skip_group_check=True)

                with tc.tile_pool(name="psm", bufs=1, space="PSUM") as psmp:
                    def amix_chunk(c, slot):
                        csl = slice(c * 128, (c + 1) * 128)
                        pat = psmp.tile([128, R], F32, tag=f"pat{slot}",
                                        name=f"pat{slot}")
                        nc.tensor.matmul(pat[:], a_sb[0][:, csl], m_sb[0][:],
                                         start=True, stop=False)
                        nc.tensor.matmul(pat[:], a_sb[1][:, csl], m_sb[1][:],
                                         start=False, stop=True)
                        nc.vector.tensor_copy(amixT[c][:], pat[:])

                    def bmix_chunk(c4):
                        sl = slice(c4 * 512, (c4 + 1) * 512)
                        pmix = psmp.tile([R, 512], F32, tag="pmix")
                        nc.tensor.matmul(pmix[:], m_sb[0][:], b_sb[0][:, sl],
                                         start=True, stop=False)
                        nc.tensor.matmul(pmix[:], m_sb[1][:], b_sb[1][:, sl],
                                         start=False, stop=True)
                        nc.vector.tensor_copy(bmix2[0:R, sl], pmix[:])

                    # JIT mixing + mm1 half A, one chunk ahead
                    amix_chunk(_ARRIVE_A[0], 0)
                    for idx in range(NCH - 1):
                        amix_chunk(_ARRIVE_A[idx + 1], (idx + 1) % 2)
                        mm1_chunk(0, _ARRIVE_A[idx], first=(idx == 0),
                                  last=False)
                    mm1_chunk(0, _ARRIVE_A[NCH - 1], first=False, last=True)
                    nc.vector.tensor_copy(zt2[0][:], zt_ps[0][:])
                    # B_mixedT after half-A mm1 (b_flat lands mid-phase-A)
                    for c4 in range(4):
                        bmix_chunk(c4)
                    nc.vector.tensor_copy(bmix2[64:64 + R, :], bmix2[0:R, :])

                with tc.tile_pool(name="psd", bufs=3, space="PSUM") as psdp:
                    def unit(st, hp):
                        sc = st // 4
                        po = 64 * (sc % 2)
                        scol = slice((st % 4) * 128, (st % 4) * 128 + 128)
                        dps = psdp.tile([128, 1024], F32, tag="dps")
                        for q in range(2):
                            osl = slice((2 * hp + q) * 512,
                                        (2 * hp + q) * 512 + 512)
                            nc.tensor.matmul(
                                dps[:, q * 512:(q + 1) * 512],
                                zt2[sc // 2][po:po + R, scol],
                                bmix2[po:po + R, osl])
                        return dps

                    def half_pair(stA, stB, hp):
                        # each [128,1024] half is cast by ONE engine and
                        # stored immediately — the store waits a single
                        # cast, emitted right before it
                        dpsA = unit(stA, hp)
                        dpsB = unit(stB, hp)
                        dcol = slice(hp * 1024, (hp + 1) * 1024)
                        dA = dp.tile([128, 1024], BF16, tag="d", name="dA")
                        dB = dp.tile([128, 1024], BF16, tag="d", name="dB")
                        if hp == 0:
                            nc.vector.tensor_copy(dA[:], dpsA[:])
                            nc.scalar.copy(dB[:], dpsB[:])
                        else:
                            nc.scalar.copy(dA[:], dpsA[:])
                            nc.vector.tensor_copy(dB[:], dpsB[:])
                        nc.sync.dma_start(
                            out_d[stA * 128:stA * 128 + 128, dcol], dA[:])
                        nc.sync.dma_start(
                            out_d[stB * 128:stB * 128 + 128, dcol], dB[:])

                    # overlap: half-A delta pairs interleaved with half-B
                    # mm1 chunks (2 per half-pair) as the B tiles stream
                    for g in range(4):
                        for hp in range(2):
                            half_pair(*_PAIRS_A[g], hp)
                            for j in range(2):
                                idx = 4 * g + 2 * hp + j
                                mm1_chunk(1, _ARRIVE_B[idx],
                                          first=(idx == 0),
                                          last=(idx == NCH - 1))
                    nc.vector.tensor_copy(zt2[1][:], zt_ps[1][:])
                    for g in range(4):
                        for hp in range(2):
                            half_pair(*_PAIRS_B[g], hp)

    nc.compile()
    return nc


def _in_maps(h, alpha, A_bank, B_bank):
    a_flat = np.ascontiguousarray(
        A_bank.reshape(KR, DIN)).astype(ml_dtypes.bfloat16)
    bt_flat = np.ascontiguousarray(
        B_bank.transpose(0, 2, 1).reshape(KR, DOUT)).astype(ml_dtypes.bfloat16)
    eye = np.eye(R, dtype=np.float32)
    maps = []
    for b in range(B):
        mix = np.kron(alpha[b].astype(np.float32).reshape(K, 1),
                      eye).astype(ml_dtypes.bfloat16)
        hT = np.ascontiguousarray(
            np.asarray(h[b]).T).astype(ml_dtypes.bfloat16)
        maps.append({
            "hbT": hT,
            "mix": np.ascontiguousarray(mix),
            "a_flat": a_flat,
            "bt_flat": bt_flat,
        })
    return maps


def _run(inputs, trace=False):
    if "nc" not in _cache:
        _cache["nc"] = _build_nc()
    nc = _cache["nc"]
    maps = _in_maps(inputs["h"], inputs["alpha"], inputs["A_bank"],
                    inputs["B_bank"])
    res = run_bass_kernel_spmd(nc, maps, core_ids=list(range(B)), trace=trace)
    out = np.stack([res.results[b]["delta"] for b in range(B)], axis=0)
    return out.astype(np.float32), res


def kernel(**inputs):
    out, _ = _run(inputs, trace=False)
    return out
